# revision 1
# baseline (speedup 1.0000x reference)
"""Trainium2 Bass kernel for nn_GAT_Comm (2-layer GAT + MLP head).

Sharding: pure data-parallel over batch B=32 across 8 NeuronCores
(4 graphs per core). Weights replicated.

Math notes (validated vs jax reference):
  exp(leaky_relu_a(s_i + d_j)) == max(exp(s_i)exp(d_j), exp(a*s_i)exp(a*d_j))
so the masked-softmax numerator is a max of two rank-1 products times the
{0,1} adjacency mask; no NxN exp pass is needed. The softmax normalizer Z
comes from a ones column appended to the aggregation matmul rhs, landing
per-partition. Attention tiles run in bf16 (adds ~3e-4 absmax error vs a
~7.5 absmax output range); the MLP runs in f32 (f32r-tagged matmuls with
N=512 moving streams).
"""

import sys

import numpy as np

sys.path.insert(0, "/opt/trn_rl_repo")

import ml_dtypes  # noqa: E402

B, N, IN, HID, HEADS, OUT = 32, 512, 128, 32, 4, 128
NEG_SLOPE = 0.2
NCORES = 8
BPC = B // NCORES  # graphs per core
P = 128  # partitions
NC4 = N // P  # 4 node chunks of 128

USE_F32R = False  # f32r matmul tag: 1 cyc/row at N>=512 vs 4 for plain f32

_cache = {}


def _patch_act_tables():
    """Steer Exp+Ln to the combined natural_log_exp set: remove exp/ln from
    the single-func sets so the table chooser can't alternate between them.
    Set order (and thus act_func_set_id indices) is preserved."""
    from concourse import bacc
    if getattr(bacc, "_ant_act_tables_patched", False):
        return
    orig = bacc.get_activation_tables

    def patched(arch):
        tabs = orig(arch)
        out = {}
        for name, fns in tabs.items():
            fns = set(fns)
            if name == "exp_and_others":
                fns = {f for f in fns if f.name != "Exp"}
            if name == "natural_log":
                fns = {f for f in fns if f.name != "Ln"}
            out[name] = fns
        return out

    bacc.get_activation_tables = patched
    bacc._ant_act_tables_patched = True


def _register_custom_op():
    """Register PQRT_MAX: out = max(in0*s0, in1*s1) (one DVE pass)."""
    import re
    from concourse import dve_ops
    from concourse.dve_spec import Spec, Src0, Src1, C0, C1, maxx
    if any(op.name == "PQRT_MAX" for op in dve_ops.OPS):
        return next(op for op in dve_ops.OPS if op.name == "PQRT_MAX")
    import numpy as np
    op = dve_ops.DveOp(
        "PQRT_MAX",
        Spec(body=maxx(Src0 * C0, Src1 * C1),
             reference=lambda in0, in1, s0, s1, imm2: np.maximum(
                 np.asarray(in0, np.float32) * s0,
                 np.asarray(in1, np.float32) * s1)),
        subdim=False,
        uops_sha={},
        perf_en={"v3": True, "v4": True},
    )
    dve_ops.OPS.append(op)
    dve_ops.CUSTOM_DVE_SPECS[op.name] = op.spec
    dve_ops._SUB_OPCODE_FOR_NAME[op.name] = (
        dve_ops._CUSTOM_DVE_ROW_BASE + len(dve_ops.OPS) - 1)
    for ver in ("v3", "v4"):
        try:
            op.compile(ver)
        except ValueError as e:
            m = re.search(r'\(%s: ([0-9a-f]+)' % ver, str(e))
            assert m, e
            op.uops_sha[ver] = m.group(1)
            op.compile(ver)
    return op


def _build_program():
    import concourse.tile as tile
    from concourse import bacc, masks, mybir

    f32 = mybir.dt.float32
    f32r = mybir.dt.float32r if USE_F32R else mybir.dt.float32
    bf16 = mybir.dt.bfloat16
    AF = mybir.ActivationFunctionType
    OP = mybir.AluOpType
    AX = mybir.AxisListType

    _patch_act_tables()
    PQRT_MAX = _register_custom_op()
    act_phase = [[] for _ in range(5)]  # exp/ln/gelu ops per table phase

    nc = bacc.Bacc("TRN2", target_bir_lowering=False, debug=False,
                   num_devices=NCORES)

    # ---- DRAM I/O ----
    d_xT = nc.dram_tensor("xT", [BPC, P, N], f32, kind="ExternalInput")
    d_xn = nc.dram_tensor("xn", [BPC, N, IN], f32, kind="ExternalInput")
    d_mk = nc.dram_tensor("mk", [BPC, N, N], bf16, kind="ExternalInput")
    d_w0 = nc.dram_tensor("w0", [P, P], f32, kind="ExternalInput")
    d_ae0 = nc.dram_tensor("ae0", [P, 2 * HEADS], bf16, kind="ExternalInput")
    d_w1 = nc.dram_tensor("w1", [P, P], bf16, kind="ExternalInput")
    d_ae1 = nc.dram_tensor("ae1", [P, 2], bf16, kind="ExternalInput")
    d_negs1 = nc.dram_tensor("negs1", [P, 1], f32, kind="ExternalInput")
    d_linw = nc.dram_tensor("linw", [2, P, OUT], f32r, kind="ExternalInput")
    d_linb = nc.dram_tensor("linb", [P, 1], f32, kind="ExternalInput")
    d_mew0 = nc.dram_tensor("mew0", [P, OUT], f32r, kind="ExternalInput")
    d_meb0 = nc.dram_tensor("meb0", [P, 1], f32, kind="ExternalInput")
    d_mew1 = nc.dram_tensor("mew1", [P, OUT], f32r, kind="ExternalInput")
    d_meb1 = nc.dram_tensor("meb1", [P, 1], f32, kind="ExternalInput")
    d_ohw = nc.dram_tensor("ohw", [P, OUT], f32r, kind="ExternalInput")
    d_ohb = nc.dram_tensor("ohb", [P, 1], f32, kind="ExternalInput")
    d_out = nc.dram_tensor("out", [BPC, N, OUT], f32, kind="ExternalOutput")

    r32 = lambda ap: ap

    with tile.TileContext(nc) as tc, \
            tc.tile_pool(name="const", bufs=1) as cpool, \
            tc.tile_pool(name="work", bufs=2) as wpool, \
            tc.tile_pool(name="abuf", bufs=4) as apool, \
            tc.tile_pool(name="stash", bufs=1) as spool, \
            tc.tile_pool(name="ps_mat", bufs=2, space="PSUM") as pmat, \
            tc.tile_pool(name="ps_misc", bufs=2, space="PSUM") as pmisc, \
            tc.tile_pool(name="ps_bf", bufs=2, space="PSUM") as pbf, \
            tc.tile_pool(name="ps_u", bufs=2, space="PSUM") as pu:

        # ---- constants ----
        id_bf = cpool.tile([P, P], bf16, tag="id_bf")
        masks.make_identity(nc, id_bf[:])
        id_f32 = cpool.tile([P, P], f32, tag="id_f32")
        masks.make_identity(nc, id_f32[:])
        eps_sb = cpool.tile([P, 1], f32, tag="eps")
        nc.gpsimd.memset(eps_sb[:], 1e-5)

        def load_const(dram_ap, shape, dtype, tag):
            t = cpool.tile(shape, dtype, tag=tag)
            nc.sync.dma_start(out=t[:], in_=dram_ap)
            return t

        w0_sb = load_const(d_w0[:], [P, P], f32, "w0")
        ae0_sb = load_const(d_ae0[:], [P, 2 * HEADS], bf16, "ae0")
        w1_sb = load_const(d_w1[:], [P, P], bf16, "w1")
        ae1_sb = load_const(d_ae1[:], [P, 2], bf16, "ae1")
        negs1_sb = load_const(d_negs1[:], [P, 1], f32, "negs1")
        linw_sb = load_const(d_linw.ap().rearrange("c k f -> k c f"),
                             [P, 2, OUT], f32r, "linw")
        linb_sb = load_const(d_linb[:], [P, 1], f32, "linb")
        mew0_sb = load_const(d_mew0[:], [P, OUT], f32r, "mew0")
        meb0_sb = load_const(d_meb0[:], [P, 1], f32, "meb0")
        mew1_sb = load_const(d_mew1[:], [P, OUT], f32r, "mew1")
        meb1_sb = load_const(d_meb1[:], [P, 1], f32, "meb1")
        ohw_sb = load_const(d_ohw[:], [P, OUT], f32r, "ohw")
        ohb_sb = load_const(d_ohb[:], [P, 1], f32, "ohb")

        # cross-phase stashes (per graph k)
        st_g2 = [spool.tile([P, NC4 * OUT], f32, tag=f"g2_{k}", name=f"g2_{k}") for k in range(BPC)]
        st_xn = [spool.tile([P, NC4 * IN], f32, tag=f"xn_{k}", name=f"xn_{k}") for k in range(BPC)]
        st_mT = [spool.tile([P, N], f32r, tag=f"mT_{k}", name=f"mT_{k}") for k in range(BPC)]
        st_e1 = [spool.tile([P, N], f32, tag=f"e1_{k}", name=f"e1_{k}") for k in range(BPC)]
        st_res = [spool.tile([P, NC4 * OUT], f32, tag=f"res_{k}", name=f"res_{k}") for k in range(BPC)]
        st_s2 = [spool.tile([P, 2 * NC4], f32, tag=f"s2_{k}", name=f"s2_{k}") for k in range(BPC)]
        st_oh = [spool.tile([P, N], f32, tag=f"oh_{k}", name=f"oh_{k}") for k in range(BPC)]
        st_go = [spool.tile([P, NC4 * OUT], f32, tag=f"go_{k}", name=f"go_{k}") for k in range(BPC)]
        st_s3 = [spool.tile([P, 2 * NC4], f32, tag=f"s3_{k}", name=f"s3_{k}") for k in range(BPC)]

        # ---------- helpers ----------
        def ln_stats(view, sums_ap, sumsq_ap):
            """view: [P, NC4, F]; per-chunk sums/sumsq [P, NC4]."""
            nc.vector.tensor_reduce(sums_ap, view, AX.X, OP.add)
            sq = wpool.tile([P, NC4 * view.shape[2]], f32, tag="sq")
            sqv = sq[:].rearrange("p (c f) -> p c f", c=NC4)
            nc.vector.tensor_tensor(sqv, view, view, OP.mult)
            nc.vector.tensor_reduce(sumsq_ap, sqv, AX.X, OP.add)

        def ln_musig(sums_ap, sumsq_ap, nfeat, phase):
            mu = wpool.tile([P, NC4], f32, tag="mu", bufs=5)
            nc.vector.tensor_scalar(mu[:], sums_ap, 1.0 / nfeat, None, OP.mult)
            musq = wpool.tile([P, NC4], f32, tag="musq", bufs=5)
            nc.vector.tensor_tensor(musq[:], mu[:], mu[:], OP.mult)
            var = wpool.tile([P, NC4], f32, tag="var", bufs=5)
            nc.vector.scalar_tensor_tensor(var[:], sumsq_ap, 1.0 / nfeat,
                                           musq[:], OP.mult, OP.subtract)
            lnv = wpool.tile([P, NC4], f32, tag="lnv", bufs=5)
            act_phase[phase].append(
                nc.scalar.activation(lnv[:], var[:], AF.Ln, bias=eps_sb[:, 0:1]))
            rstd = wpool.tile([P, NC4], f32, tag="rstd", bufs=5)
            act_phase[phase].append(
                nc.scalar.activation(rstd[:], lnv[:], AF.Exp, scale=-0.5))
            return mu, rstd

        def gat_prep(hT_bf_ap, nh, ae_sb, lay, k):
            """Score matmuls + exp factors for one graph/layer."""
            es_ps = pmisc.tile([nh, N], f32, tag="misc")
            nc.tensor.matmul(es_ps[:], ae_sb[:, 0:nh], hT_bf_ap,
                             start=True, stop=True)
            ed_ps = pmisc.tile([nh, N], f32, tag="misc")
            nc.tensor.matmul(ed_ps[:], ae_sb[:, nh:2 * nh], hT_bf_ap,
                             start=True, stop=True)
            pq = wpool.tile([nh, N], bf16, tag=f"pq{lay}_{k}", bufs=1)
            rt = wpool.tile([nh, N], bf16, tag=f"rt{lay}_{k}", bufs=1)
            act_phase[0].append(nc.scalar.activation(pq[:], es_ps[:], AF.Exp))
            act_phase[0].append(
                nc.scalar.activation(rt[:], es_ps[:], AF.Exp, scale=NEG_SLOPE))
            ed_sb = wpool.tile([nh, N], f32, tag=f"ed{lay}")
            nc.vector.tensor_copy(ed_sb[:], ed_ps[:])
            dcol_ps = pmisc.tile([P, NC4 * nh], f32, tag="misc")
            for c in range(NC4):
                nc.tensor.transpose(dcol_ps[:, c * nh:(c + 1) * nh],
                                    ed_sb[:, c * P:(c + 1) * P],
                                    id_f32[0:nh, 0:nh])
            qcol = wpool.tile([P, NC4 * nh], f32, tag=f"qc{lay}_{k}", bufs=1)
            tcol = wpool.tile([P, NC4 * nh], f32, tag=f"tc{lay}_{k}", bufs=1)
            act_phase[0].append(nc.scalar.activation(qcol[:], dcol_ps[:], AF.Exp))
            act_phase[0].append(
                nc.scalar.activation(tcol[:], dcol_ps[:], AF.Exp, scale=NEG_SLOPE))

            if nh > 1:
                # gpsimd reads must start at partition 0/32/64/96: spread the
                # per-head rows onto quarter partitions via SBUF->SBUF DMA
                pq4 = wpool.tile([P, N], bf16, tag=f"pq4{lay}_{k}", bufs=1)
                rt4 = wpool.tile([P, N], bf16, tag=f"rt4{lay}_{k}", bufs=1)
                q4 = lambda t: t[:].rearrange("(h r) x -> h r x", r=P // nh)[:, 0, :]
                nc.sync.dma_start(out=q4(pq4), in_=pq[:])
                nc.sync.dma_start(out=q4(rt4), in_=rt[:])
            else:
                pq4, rt4 = pq, rt
            return dict(pq4=pq4, rt4=rt4, qcol=qcol, tcol=tcol)

        def gat_heads(G, nh, dh, mk_sb, aug, out_nat, lay):
            """Per-head broadcast/combine/mask/aggregate/normalize."""
            pq4, rt4, qcol, tcol = G["pq4"], G["rt4"], G["qcol"], G["tcol"]
            gs = NC4 if (dh + 1) * NC4 <= 512 else 2  # PSUM bank limit
            hstep = P // nh if nh > 1 else 0
            for h in range(nh):
                pb = apool.tile([P, N], bf16, tag="pb")
                rb = apool.tile([P, N], bf16, tag="rb")
                nc.gpsimd.partition_broadcast(pb[:], pq4[h * hstep:h * hstep + 1, :])
                nc.gpsimd.partition_broadcast(rb[:], rt4[h * hstep:h * hstep + 1, :])
                a_sb = apool.tile([P, NC4 * N], bf16, tag="a_sb")
                for c in range(NC4):
                    sl = slice(c * N, (c + 1) * N)
                    nc.vector._custom_dve(
                        PQRT_MAX, out=a_sb[:, sl], in0=pb[:], in1=rb[:],
                        s0=qcol[:, c * nh + h:c * nh + h + 1],
                        s1=tcol[:, c * nh + h:c * nh + h + 1])
                # one fused mask multiply over all 4 chunks
                eng = nc.gpsimd if (lay == 2 or h % 2 == 1) else nc.vector
                eng.tensor_tensor(a_sb[:], a_sb[:],
                                  mk_sb[:].rearrange("j c i -> j (c i)"),
                                  OP.mult)
                for g0 in range(0, NC4, gs):
                    u_ps = pu.tile([P, gs * (dh + 1)], f32, tag="u_ps")
                    for i in range(gs):
                        ic = g0 + i
                        for jc in range(NC4):
                            nc.tensor.matmul(
                                u_ps[:, i * (dh + 1):(i + 1) * (dh + 1)],
                                a_sb[:, jc * N + ic * P: jc * N + (ic + 1) * P],
                                aug[jc][:, h * (dh + 1):(h + 1) * (dh + 1)],
                                start=(jc == 0), stop=(jc == NC4 - 1))
                    rz = wpool.tile([P, gs], f32, tag="rz")
                    uv = u_ps[:].rearrange("p (c u) -> p c u", c=gs)
                    nc.vector.reciprocal(rz[:], uv[:, :, dh])
                    rzb = rz[:].rearrange("p (c o) -> p c o", o=1)\
                        .to_broadcast((P, gs, dh))
                    onv = out_nat.rearrange("p (c f) -> p c f", c=NC4)
                    nc.vector.tensor_tensor(
                        onv[:, g0:g0 + gs, h * dh:(h + 1) * dh],
                        uv[:, :, 0:dh], rzb, OP.mult)

        def transpose_back(srcT_ap, dst_ap):
            """[P, N] f32 T-form -> natural via 4 PE transposes."""
            for ic in range(NC4):
                tp = pmisc.tile([P, P], f32, tag="misc")
                nc.tensor.transpose(tp[:], srcT_ap[:, ic * P:(ic + 1) * P],
                                    id_f32[:])
                nc.scalar.copy(dst_ap[:, ic * P:(ic + 1) * P], tp[:])

        # =======================================================
        # PASS 1 (exp/ln table), software-pipelined stage-major over k:
        #   A: loads + h prep + L1 score prep
        #   B: L1 heads   C: elu + L2 prep   D: L2 head   E: ln1+lin+me0
        # =======================================================
        per_k = [dict() for _ in range(BPC)]

        def stageA(k):
            S = per_k[k]
            xt_sb = wpool.tile([P, N], f32, tag="xt")
            nc.sync.dma_start(out=xt_sb[:], in_=d_xT[k, :, :])
            mk_sb = wpool.tile([P, NC4, N], bf16, tag=f"mk_{k}", bufs=1)
            nc.sync.dma_start(
                out=mk_sb[:],
                in_=d_mk[k].rearrange("(c j) i -> j c i", j=P))
            nc.sync.dma_start(
                out=st_xn[k][:].rearrange("p (c f) -> p c f", c=NC4),
                in_=d_xn[k].rearrange("(c p) f -> p c f", p=P))
            S["mk"] = mk_sb
            S["done"] = True

            hT_ps = pmat.tile([P, N], f32, tag="mat")
            nc.tensor.matmul(hT_ps[:], w0_sb[:], xt_sb[:], start=True, stop=True)
            hT_bf = wpool.tile([P, N], bf16, tag="hT_bf")
            nc.vector.tensor_copy(hT_bf[:], hT_ps[:])
            aug1 = []
            for c in range(NC4):
                hp = pbf.tile([P, P], bf16, tag="hpbf")
                nc.tensor.transpose(hp[:], hT_bf[:, c * P:(c + 1) * P], id_bf[:])
                ha = wpool.tile([P, HEADS * (HID + 1)], bf16, tag=f"ha{c}_{k}", bufs=1)
                hav = ha[:].rearrange("p (h d) -> p h d", h=HEADS)
                nc.scalar.copy(
                    hav[:, :, 0:HID],
                    hp[:].rearrange("p (h d) -> p h d", h=HEADS))
                nc.gpsimd.memset(hav[:, :, HID:HID + 1], 1.0)
                aug1.append(ha)
            S["aug1"] = aug1
            S["G1"] = gat_prep(hT_bf[:], HEADS, ae0_sb, 1, k)

        def stageB(k):
            S = per_k[k]
            mn = wpool.tile([P, N], f32, tag=f"mn_{k}", bufs=1)
            gat_heads(S["G1"], HEADS, HID, S["mk"], S["aug1"], mn[:], 1)
            S["mn"] = mn

        def stageC(k):
            S = per_k[k]
            mn = S["mn"]
            t0 = wpool.tile([P, N], f32, tag="t0")
            nc.vector.tensor_scalar(t0[:], mn[:], 0.0, None, OP.min)
            t1 = wpool.tile([P, N], f32, tag="t1")
            act_phase[0].append(nc.scalar.activation(t1[:], t0[:], AF.Exp))
            melu = wpool.tile([P, N], bf16, tag="melu")
            nc.vector.scalar_tensor_tensor(melu[:], mn[:], 0.0, t1[:],
                                           OP.max, OP.add)
            meluT = wpool.tile([P, N], bf16, tag="meluT")
            for c in range(NC4):
                tp = pbf.tile([P, P], bf16, tag="hpbf")
                nc.tensor.transpose(tp[:], melu[:, c * P:(c + 1) * P], id_bf[:])
                nc.scalar.copy(meluT[:, c * P:(c + 1) * P], tp[:])

            h2T_ps = pmat.tile([P, N], f32, tag="mat")
            nc.tensor.matmul(h2T_ps[:], w1_sb[:], meluT[:], start=True, stop=True)
            h2T_bf = wpool.tile([P, N], bf16, tag="h2T_bf")
            nc.vector.tensor_scalar(h2T_bf[:], h2T_ps[:], negs1_sb[:, 0:1],
                                    None, OP.add)
            aug2 = []
            for c in range(NC4):
                hp = pbf.tile([P, P], bf16, tag="hpbf")
                nc.tensor.transpose(hp[:], h2T_bf[:, c * P:(c + 1) * P], id_bf[:])
                ha = wpool.tile([P, OUT + 1], bf16, tag=f"h2a{c}_{k}", bufs=1)
                nc.scalar.copy(ha[:, 0:OUT], hp[:])
                nc.gpsimd.memset(ha[:, OUT:OUT + 1], 1.0)
                aug2.append(ha)
            S["aug2"] = aug2
            S["G2"] = gat_prep(h2T_bf[:], 1, ae1_sb, 2, k)

        def stageD(k):
            S = per_k[k]
            gat_heads(S["G2"], 1, OUT, S["mk"], S["aug2"], st_g2[k][:], 2)

        def stageE(k):
            xv = st_xn[k][:].rearrange("p (c f) -> p c f", c=NC4)
            gv = st_g2[k][:].rearrange("p (c f) -> p c f", c=NC4)
            r1 = wpool.tile([P, NC4], f32, tag="r1")
            r2 = wpool.tile([P, NC4], f32, tag="r2")
            s1 = wpool.tile([P, NC4], f32, tag="s1")
            s2 = wpool.tile([P, NC4], f32, tag="s2")
            ln_stats(xv, r1[:], s1[:])
            ln_stats(gv, r2[:], s2[:])
            nc.vector.tensor_tensor(r1[:], r1[:], r2[:], OP.add)
            nc.vector.tensor_tensor(s1[:], s1[:], s2[:], OP.add)
            mu, rstd = ln_musig(r1[:], s1[:], IN + OUT, 0)

            catT = [wpool.tile([P, N], f32r, tag=f"catT{i}", name=f"catT{i}")
                    for i in range(2)]
            for src_v, dstT in ((xv, catT[0]), (gv, catT[1])):
                for ic in range(NC4):
                    tn = wpool.tile([P, P], f32, tag="tn")
                    nc.vector.tensor_scalar(tn[:], src_v[:, ic, :],
                                            mu[:, ic:ic + 1], rstd[:, ic:ic + 1],
                                            OP.subtract, OP.mult)
                    tp = pmisc.tile([P, P], f32, tag="misc")
                    nc.tensor.transpose(tp[:], tn[:], id_f32[:])
                    nc.scalar.copy(dstT[:, ic * P:(ic + 1) * P], tp[:])

            mT_ps = pmat.tile([P, N], f32, tag="mat")
            nc.tensor.matmul(mT_ps[:], r32(linw_sb[:, 0, :]), r32(catT[0][:]),
                             start=True, stop=False)
            nc.tensor.matmul(mT_ps[:], r32(linw_sb[:, 1, :]), r32(catT[1][:]),
                             start=False, stop=True)
            nc.vector.tensor_scalar(st_mT[k][:], mT_ps[:], linb_sb[:, 0:1],
                                    None, OP.add)
            e1_ps = pmat.tile([P, N], f32, tag="mat")
            nc.tensor.matmul(e1_ps[:], r32(mew0_sb[:]), r32(st_mT[k][:]),
                             start=True, stop=True)
            nc.vector.tensor_copy(st_e1[k][:], e1_ps[:])

        for stage in (stageA, stageB, stageC, stageD, stageE):
            for k in range(BPC):
                stage(k)

        # =======================================================
        # PASS 2 (gelu table): gelu(me0), me1, residual, ln2 stats
        # =======================================================
        for k in range(BPC):
            gT = wpool.tile([P, N], f32r, tag="gT")
            act_phase[1].append(
                nc.scalar.activation(gT[:], st_e1[k][:], AF.Gelu,
                                     bias=meb0_sb[:, 0:1]))
            encT_ps = pmat.tile([P, N], f32, tag="mat")
            nc.tensor.matmul(encT_ps[:], r32(mew1_sb[:]), r32(gT[:]),
                             start=True, stop=True)
            resT = wpool.tile([P, N], f32, tag="resT")
            nc.vector.tensor_scalar(resT[:], encT_ps[:], meb1_sb[:, 0:1],
                                    None, OP.add)
            nc.vector.tensor_tensor(resT[:], resT[:], st_mT[k][:], OP.add)
            transpose_back(resT[:], st_res[k][:])
            rv = st_res[k][:].rearrange("p (c f) -> p c f", c=NC4)
            ln_stats(rv, st_s2[k][:, 0:NC4], st_s2[k][:, NC4:2 * NC4])

        # =======================================================
        # PASS 3 (exp/ln): ln2 -> oh matmul  (rstd batched across k)
        # =======================================================
        mus2, rstds2 = [], []
        for k in range(BPC):
            mu, rstd = ln_musig(st_s2[k][:, 0:NC4], st_s2[k][:, NC4:2 * NC4],
                                OUT, 2)
            mus2.append(mu); rstds2.append(rstd)
        for k in range(BPC):
            mu, rstd = mus2[k], rstds2[k]
            ln2T = wpool.tile([P, N], f32r, tag="ln2T")
            rv = st_res[k][:].rearrange("p (c f) -> p c f", c=NC4)
            for ic in range(NC4):
                tn = wpool.tile([P, P], f32, tag="tn")
                nc.vector.tensor_scalar(tn[:], rv[:, ic, :], mu[:, ic:ic + 1],
                                        rstd[:, ic:ic + 1], OP.subtract, OP.mult)
                tp = pmisc.tile([P, P], f32, tag="misc")
                nc.tensor.transpose(tp[:], tn[:], id_f32[:])
                nc.scalar.copy(ln2T[:, ic * P:(ic + 1) * P], tp[:])
            ohT_ps = pmat.tile([P, N], f32, tag="mat")
            nc.tensor.matmul(ohT_ps[:], r32(ohw_sb[:]), r32(ln2T[:]),
                             start=True, stop=True)
            nc.vector.tensor_copy(st_oh[k][:], ohT_ps[:])

        # =======================================================
        # PASS 4 (gelu): gelu(oh) -> natural, ln3 stats
        # =======================================================
        for k in range(BPC):
            goT = wpool.tile([P, N], f32, tag="goT")
            act_phase[3].append(
                nc.scalar.activation(goT[:], st_oh[k][:], AF.Gelu,
                                     bias=ohb_sb[:, 0:1]))
            transpose_back(goT[:], st_go[k][:])
            gv = st_go[k][:].rearrange("p (c f) -> p c f", c=NC4)
            ln_stats(gv, st_s3[k][:, 0:NC4], st_s3[k][:, NC4:2 * NC4])

        # =======================================================
        # PASS 5 (exp/ln): ln3 + output DMA
        # =======================================================
        mus3, rstds3 = [], []
        for k in range(BPC):
            mu, rstd = ln_musig(st_s3[k][:, 0:NC4], st_s3[k][:, NC4:2 * NC4],
                                OUT, 4)
            mus3.append(mu); rstds3.append(rstd)
        for k in range(BPC):
            mu, rstd = mus3[k], rstds3[k]
            ov = wpool.tile([P, NC4, OUT], f32, tag="ov")
            gv = st_go[k][:].rearrange("p (c f) -> p c f", c=NC4)
            for ic in range(NC4):
                nc.vector.tensor_scalar(ov[:, ic, :], gv[:, ic, :],
                                        mu[:, ic:ic + 1], rstd[:, ic:ic + 1],
                                        OP.subtract, OP.mult)
            nc.sync.dma_start(
                out=d_out[k].rearrange("(c p) f -> p c f", p=P),
                in_=ov[:])

        from concourse.tile_rust import add_dep_helper
        for ph in range(4):
            for a in act_phase[ph]:
                for b in act_phase[ph + 1]:
                    add_dep_helper(b.ins, a.ins, sync=False,
                                   reason="act-table-phase-order")

    nc.compile()
    return nc


def _prep_inputs(x, graph, W0, a_src0, a_dst0, W1, a_src1, a_dst1,
                 ln1_g, ln1_b, lin_W, lin_b, me_W0, me_b0, me_W1, me_b1,
                 ln2_g, ln2_b, oh_W, oh_b, ln3_g, ln3_b):
    bf16 = ml_dtypes.bfloat16
    x = np.ascontiguousarray(x, dtype=np.float32)
    xT = np.ascontiguousarray(x.transpose(0, 2, 1))
    eye = np.eye(N, dtype=bool)
    mask = (graph > 0) | eye
    mkT = np.ascontiguousarray(mask.transpose(0, 2, 1)).astype(bf16)

    w0 = np.ascontiguousarray(W0.reshape(IN, HEADS * HID), dtype=np.float32)
    ae0 = np.zeros((P, 2 * HEADS), np.float32)
    for h in range(HEADS):
        ae0[h * HID:(h + 1) * HID, h] = a_src0[h]
        ae0[h * HID:(h + 1) * HID, HEADS + h] = a_dst0[h]
    w1 = np.ascontiguousarray(W1.reshape(P, OUT), dtype=np.float32)
    ae1 = np.zeros((P, 2), np.float32)
    ae1[:, 0] = a_src1[0]
    ae1[:, 1] = a_dst1[0]
    w1_bf = w1.astype(bf16)
    # elu fold: kernel computes W1^T @ (elu+1); subtract colsums of (bf16) W1
    negs1 = -w1_bf.astype(np.float32).sum(axis=0).reshape(P, 1)

    linw_eff = (ln1_g[:, None] * lin_W).astype(np.float32)
    linb_eff = (ln1_b @ lin_W + lin_b).astype(np.float32)
    ohw_eff = (ln2_g[:, None] * oh_W).astype(np.float32)
    ohb_eff = (ln2_b @ oh_W + oh_b).astype(np.float32)
    assert np.allclose(ln3_g, 1) and np.allclose(ln3_b, 0), \
        "nontrivial ln3 affine not supported by this kernel build"

    common = {
        "w0": w0,
        "ae0": ae0.astype(bf16),
        "w1": w1_bf,
        "ae1": ae1.astype(bf16),
        "negs1": np.ascontiguousarray(negs1),
        "linw": np.ascontiguousarray(linw_eff.reshape(2, P, OUT)),
        "linb": np.ascontiguousarray(linb_eff.reshape(P, 1)),
        "mew0": np.ascontiguousarray(me_W0, dtype=np.float32),
        "meb0": np.ascontiguousarray(np.asarray(me_b0, np.float32).reshape(P, 1)),
        "mew1": np.ascontiguousarray(me_W1, dtype=np.float32),
        "meb1": np.ascontiguousarray(np.asarray(me_b1, np.float32).reshape(P, 1)),
        "ohw": np.ascontiguousarray(ohw_eff),
        "ohb": np.ascontiguousarray(ohb_eff.reshape(P, 1)),
    }
    in_maps = []
    for c in range(NCORES):
        sl = slice(c * BPC, (c + 1) * BPC)
        in_maps.append({
            "xT": np.ascontiguousarray(xT[sl]),
            "xn": np.ascontiguousarray(x[sl]),
            "mk": np.ascontiguousarray(mkT[sl]),
            **common,
        })
    return in_maps


def kernel(**inputs) -> np.ndarray:
    from concourse.bass_utils import run_bass_kernel_spmd

    if "nc" not in _cache:
        _cache["nc"] = _build_program()
    nc = _cache["nc"]
    in_maps = _prep_inputs(**{k: np.asarray(v) for k, v in inputs.items()})
    res = run_bass_kernel_spmd(nc, in_maps, core_ids=list(range(NCORES)))
    out = np.concatenate([r["out"] for r in res.results], axis=0)
    return np.ascontiguousarray(out, dtype=np.float32)

